# revision 31
# baseline (speedup 1.0000x reference)
"""Trainium2 Bass kernel for nn_MoEsparseRoutingForClassification.

Reference computation (B=64, S=128, H=1024, E=8, L=2):
    x = X[:, 0, :]                                   # CLS token [B,H]
    y[b,o]   = sum_e g[b,e] * (x[b] . dense_w[e,o,:]) + (g @ dense_b)[b,o]
    t        = tanh(y)
    out[b,l] = sum_e g[b,e] * (t[b] . out_w[e,l,:])  + (g @ out_b)[b,l]

Distribution: the H output dim of the dense layer is sharded 8 ways
(OC=128 per core).  Core c computes y[:, c*OC:(c+1)*OC] (which needs the
full CLS token but only a slice dense_w[:, c_slice, :]), applies tanh,
and contracts its slice against out_w[:, :, c_slice] to produce a
partial [L, 128] logit tile.  The partials (incl. the out_b bias, fed
only to core 0) sum to the full output on the host.  No cross-core
collective is needed.

v3 notes (trace-driven):
- w1/xt ship as bf16 (host cast): halves the dominant HBM train and makes
  stage-1 matmuls single-pass.  Measured rel err vs fp32 ref: ~2.3e-3
  (tolerance 2e-2).
- gc (gate columns + a 128x128 fp32 identity) goes FIRST on the sync ring:
  small transfers issued after the w1 flood starve at ~1.5% BW (packet
  round-robin), which starved the gate-broadcast build in v2.
- ~18 dummy warm-up matmuls run during the DMA train so HAM un-throttles
  the PE (cold MMs run at 1.2 GHz, 2x slower).  They write psum_y with
  start&stop so the real k=0 (start=True) ordering is enforced via WAW.
- Output is produced TRANSPOSED as [L, 128] on 2 partitions: the [128, 2]
  layout needed 128 8-byte HBM-write descriptors whose completion receipt
  cost ~2.5us; 2 partitions x 512B needs 2.  The transpose rides the PE
  (pre_t.T @ I) and accumulates straight onto the sel_ob partial in PSUM,
  fusing the bias add.
- enable_partition_id=False drops the per-engine partition-id loads from
  the BSP preamble.
"""

import sys

import numpy as np

for _p in ("/opt/trn_rl_repo",):
    if _p not in sys.path:
        sys.path.insert(0, _p)

# If the environment sets BASS_TRACE but lacks antenv.axon_hooks (this agent
# image does), run_bass_kernel_spmd would crash on import; pre-seed a no-op
# module so tracing degrades gracefully instead.
try:  # pragma: no cover
    import antenv.axon_hooks  # noqa: F401
except Exception:  # pragma: no cover
    import types as _types

    _m = _types.ModuleType("antenv.axon_hooks")
    _m._hook = None
    _m.set_axon_ntff_profile_hook = lambda h: setattr(_m, "_hook", h)
    _m.get_axon_ntff_profile_hook = lambda: _m._hook
    sys.modules["antenv.axon_hooks"] = _m

B, S, H = 64, 128, 1024
E, L = 8, 2
NCORES = 8
OC = H // NCORES          # dense-output slice per core (128)
HC = OC // 2              # half-slice mapped to a PSUM partition half (64)
KT = H // 128             # contraction tiles
P = 128
XT_ELEMS = KT * B         # 512 bf16 elems/partition, rides at the train head
KP = 2 * E * HC           # elems per k-plane per partition (1024)
# w1 k-plane groups per DMA chunk; xt is fused into chunk 0 so it moves in
# large packets instead of a slow 1KB-packet prefix.  Chunk 0 rides the
# SCALAR ring so both DGE rings generate descriptors concurrently (faster
# ramp) and its transfer overlaps the sync-ring chunks; k7 alone keeps the
# post-train matmul trail to one pair.  Few chunks: every DMA's semaphore
# waits on the slowest of 16 SDMA engines (~1-2.5us straggle under load).
W1_CHUNKS = ((0, 2), (2, 5), (5, 7), (7, 8))

_cached = None


def _build():
    from contextlib import ExitStack

    import concourse.tile as tile
    from concourse import bacc, mybir

    F32 = mybir.dt.float32
    BF16 = mybir.dt.bfloat16
    AF = mybir.ActivationFunctionType
    OP = mybir.AluOpType

    nc = bacc.Bacc("TRN2", target_bir_lowering=False, debug=False,
                   num_devices=NCORES, enable_partition_id=False)

    # E-pack along the free dim (one DMA for all E-partition consts):
    #   gt [E,B] | db [E,2,HC] | ow2 [E,2,L,HC] | ob [E,L] | gtz [E,P]
    EPACK = B + OC + L * OC + L + P      # 64+128+256+2+128 = 578
    wx_d = nc.dram_tensor("wx", [P, XT_ELEMS + KT * KP], BF16,
                          kind="ExternalInput")
    ep_d = nc.dram_tensor("ep", [E, EPACK], F32, kind="ExternalInput")
    gc_d = nc.dram_tensor("gc", [P, E + P + B + OC], F32,
                          kind="ExternalInput")
    out_d = nc.dram_tensor("out", [L, P], F32, kind="ExternalOutput")

    with tile.TileContext(nc) as tc, ExitStack() as ctx:
        consts = ctx.enter_context(tc.tile_pool(name="consts", bufs=1))
        wpool = ctx.enter_context(tc.tile_pool(name="wpool", bufs=1))
        mixp = ctx.enter_context(tc.tile_pool(name="mixp", bufs=1))
        smallp = ctx.enter_context(tc.tile_pool(name="smallp", bufs=1))
        psy = ctx.enter_context(tc.tile_pool(name="psy", bufs=1, space="PSUM"))
        pss = ctx.enter_context(tc.tile_pool(name="pss", bufs=1, space="PSUM"))

        # Sync ring: the wx train (xt fused into chunk 0).  Scalar ring: gc
        # then ep -- the Scalar engine's ACT_TABLE_LOAD (~1.3us) delays its
        # DMA issues, so only non-critical transfers ride there.  gc also
        # carries gates.T and the dense_b slice (rows 0-7) so the sel_db
        # matmuls don't wait for ep's straggling sem.
        wx_t = wpool.tile([P, XT_ELEMS + KT * KP], BF16)
        xt_t = wx_t[:, 0:XT_ELEMS].rearrange("p (k b) -> p k b", k=KT)
        w1_t = wx_t[:, XT_ELEMS:].rearrange(
            "p (k h e c) -> p k h e c", k=KT, h=2, e=E)
        bounds = [0] + [XT_ELEMS + khi * KP for _, khi in W1_CHUNKS]
        for lo, hi in zip(bounds[:-1], bounds[1:]):
            nc.sync.dma_start(out=wx_t[:, lo:hi], in_=wx_d.ap()[:, lo:hi])
        GCW = E + P + B + OC
        gc_t = consts.tile([P, GCW], F32)
        ident_t = gc_t[:, E:E + P]               # [128,128] fp32 identity
        gt2_t = gc_t[0:E, E + P:E + P + B]       # gates.T [E,B]
        db2_t = gc_t[0:E, E + P + B:].rearrange("e (h c) -> e h c", h=2)
        nc.scalar.dma_start(out=gc_t, in_=gc_d.ap())
        ep_t = consts.tile([E, EPACK], F32)
        nc.scalar.dma_start(out=ep_t, in_=ep_d.ap())
        o = 0
        gt_t = ep_t[:, o:o + B]; o += B
        db_t = ep_t[:, o:o + OC].rearrange("e (h c) -> e h c", h=2); o += OC
        ow_t = ep_t[:, o:o + L * OC].rearrange(
            "e (h l c) -> e h l c", h=2, l=L); o += L * OC
        ob_t = ep_t[:, o:o + L]; o += L
        gtz_t = ep_t[:, o:o + P]                 # gates.T | zeros

        psum_y = psy.tile([P, E, HC], F32)

        # ---- stage 1: y[64h+b, (e, hc)] = x . dense_w[e, oc_half, :] ----
        # The h=0 / h=1 matmuls write PSUM partition halves 0-63 / 64-127,
        # i.e. disjoint PE col-groups -> the two bf16 streams overlap.
        # k-outer so the PE consumes each w1 chunk as it lands.
        # Gate-broadcast table gb[p, (e, hc)] = g[b, e], built early on the
        # DVE (hidden under the w1 DMA stream).
        ones_t = smallp.tile([P, HC], F32)
        nc.vector.memset(ones_t[:], 1.0)
        gb_t = consts.tile([P, E, HC], F32)
        for e in range(1, E):                    # e=0 is gated via STT below
            nc.vector.tensor_scalar_mul(gb_t[:, e, :], ones_t[:],
                                        gc_t[:, e:e + 1])

        psum_db = pss.tile([P, HC], F32)
        sdb_t = smallp.tile([P, HC], F32)
        for k in range(KT):
            for h in range(2):
                nc.tensor.matmul(
                    psum_y[h * 64:h * 64 + 64, :, :].rearrange(
                        "b e c -> b (e c)"),
                    xt_t[:, k, :],
                    w1_t[:, k, h].rearrange("p e c -> p (e c)"),
                    start=(k == 0),
                    stop=(k == KT - 1),
                    skip_group_check=True,
                )
            if k == W1_CHUNKS[0][1] - 1:
                # sel_db^h [64h+b, hc] from the gc-borne gates/biases:
                # runs in the chunk-sem gap so sdb is ready well before the
                # mix tail needs it.
                for h in range(2):
                    nc.tensor.matmul(
                        psum_db[h * 64:h * 64 + 64, :], gt2_t, db2_t[:, h, :],
                        start=True, stop=True, skip_group_check=True)
                nc.scalar.copy(sdb_t[:], psum_db[:])

        # ---- remaining small matmuls AFTER the k-loop: they wait on the
        # scalar-ring ep sem and their outputs aren't needed until stage 2.
        # sel_ow^h [64h+b, (l, hc)] ; transposed sel_ob partial [l, 64h+b]
        # (gtz zeroes the h=1 copy so the host h-sum counts ob once;
        # pre.T accumulates onto it later).
        psum_ow = pss.tile([P, L, HC], F32)
        for h in range(2):
            sl = slice(h * 64, h * 64 + 64)
            nc.tensor.matmul(
                psum_ow[sl, :, :].rearrange("b l c -> b (l c)"),
                gt_t, ow_t[:, h].rearrange("e l c -> e (l c)"),
                start=True, stop=True, skip_group_check=True,
            )
        psum_oT = pss.tile([L, P], F32)
        nc.tensor.matmul(psum_oT[:], ob_t, gtz_t,
                         start=True, stop=False, skip_group_check=True)

        # gate-mix: prod in bf16 so the pairwise tree runs in 2x DVE mode.
        # The sel_db bias folds into the e=0 slice for free via STT:
        # prod[0] = y[0]*g[0] + sel_db.
        prod_t = mixp.tile([P, E, HC], BF16)
        nc.vector.tensor_tensor(
            out=prod_t[:, 1:8, :], in0=psum_y[:, 1:8, :], in1=gb_t[:, 1:8, :],
            op=OP.mult,
        )
        nc.vector.scalar_tensor_tensor(
            out=prod_t[:, 0, :], in0=psum_y[:, 0, :], scalar=gc_t[:, 0:1],
            in1=sdb_t[:], op0=OP.mult, op1=OP.add,
        )
        t1 = mixp.tile([P, 4, HC], BF16)
        nc.vector.tensor_add(t1[:], prod_t[:, 0:4, :], prod_t[:, 4:8, :])
        t2 = mixp.tile([P, 2, HC], BF16)
        nc.vector.tensor_add(t2[:], t1[:, 0:2, :], t1[:, 2:4, :])
        acc = mixp.tile([P, HC], BF16)
        nc.vector.tensor_add(acc[:], t2[:, 0, :], t2[:, 1, :])

        t_t = smallp.tile([P, HC], F32)
        nc.scalar.activation(t_t[:], acc[:], AF.Tanh)

        # ---- stage 2: pre[64h+b, l] = sum_hc t * sel_ow ----
        # NOTE: InstTensorTensorReduce faults TRN2; scalar_tensor_tensor with
        # accum_out (free-dim sum) is the reliable path.
        pre_t = smallp.tile([P, L], F32)
        dump = smallp.tile([P, HC], F32)
        for l in range(L):
            nc.vector.scalar_tensor_tensor(
                out=dump[:],
                in0=psum_ow[:, l, :],
                scalar=1.0,
                in1=t_t[:],
                op0=OP.mult,
                op1=OP.mult,
                accum_out=pre_t[:, l:l + 1],
            )
        # transpose pre onto the sel_ob partial: psum_oT += pre.T
        # (PE transpose datapath; PSUM accumulation does the add)
        nc.tensor.matmul(psum_oT[:], pre_t[:], ident_t, is_transpose=True,
                         start=False, stop=True, skip_group_check=True)
        outT_t = smallp.tile([L, P], F32)
        nc.scalar.copy(outT_t[:], psum_oT[:])

        nc.sync.dma_start(out=out_d.ap(), in_=outT_t[:])

    nc.compile()
    return nc


def _prep_inputs(X, gates, dense_w, dense_b, out_w, out_b):
    """Host-side layout prep (slice/transpose/cast) -> per-core input maps."""
    import ml_dtypes

    BF = ml_dtypes.bfloat16
    X = np.asarray(X, dtype=np.float32)
    gates = np.asarray(gates, dtype=np.float32)
    dense_w = np.asarray(dense_w, dtype=np.float32)
    dense_b = np.asarray(dense_b, dtype=np.float32)
    out_w = np.asarray(out_w, dtype=np.float32)
    out_b = np.asarray(out_b, dtype=np.float32)

    xcls = X[:, 0, :]                                     # [B, H]
    # xt[i_lo, k, b] = x[b, k*128 + i_lo]
    xt = (xcls.T.reshape(KT, P, B).transpose(1, 0, 2)
          .astype(BF).reshape(P, XT_ELEMS))
    gt = np.ascontiguousarray(gates.T)                    # [E, B]
    gtz = np.concatenate([gt, np.zeros_like(gt)], axis=1)  # [E, 128]

    in_maps = []
    for c in range(NCORES):
        sl = slice(c * OC, (c + 1) * OC)
        # w1[i_lo, k, h, e, hc] = dense_w[e, c*OC + h*64 + hc, k*128 + i_lo]
        w1 = (dense_w[:, sl, :]                 # [E, OC, H]
              .reshape(E, 2, HC, KT, P)         # [e, h, hc, k, i_lo]
              .transpose(4, 3, 1, 0, 2)         # [i_lo, k, h, e, hc]
              .astype(BF).reshape(P, KT * KP))
        wx = np.ascontiguousarray(np.concatenate([xt, w1], axis=1))

        # ow2[e, (h, l, hc)] = out_w[e, l, c*OC + h*64 + hc]
        ow2 = (out_w[:, :, sl].reshape(E, L, 2, HC)
               .transpose(0, 2, 1, 3).reshape(E, L * OC))
        ob = out_b if c == 0 else np.zeros_like(out_b)
        ep = np.ascontiguousarray(
            np.concatenate([gt, dense_b[:, sl], ow2, ob, gtz], axis=1)
        )
        # gc: gate columns (dup across the two PSUM halves) | fp32 identity
        # | gates.T and the dense_b slice packed into rows 0..E-1
        db2 = (dense_b[:, sl].reshape(E, 2, HC).reshape(E, OC))
        gtdb = np.zeros((P, B + OC), dtype=np.float32)
        gtdb[:E, :B] = gt
        gtdb[:E, B:] = db2
        gc2 = np.ascontiguousarray(np.concatenate(
            [np.vstack([gates, gates]), np.eye(P, dtype=np.float32), gtdb],
            axis=1))
        in_maps.append({
            "wx": wx,
            "ep": ep,
            "gc": gc2,
        })
    return in_maps


def _run(in_maps, trace=False, tmpdir=None):
    global _cached
    from concourse.bass_utils import run_bass_kernel_spmd

    if _cached is None:
        _cached = _build()
    res = run_bass_kernel_spmd(
        _cached, in_maps, list(range(NCORES)), trace=trace, tmpdir=tmpdir,
    )
    return res


def kernel(X, gates, dense_w, dense_b, out_w, out_b):
    in_maps = _prep_inputs(X, gates, dense_w, dense_b, out_w, out_b)
    res = _run(in_maps)
    acc = np.zeros((B, L), dtype=np.float64)
    for c in range(NCORES):
        part = res.results[c]["out"].astype(np.float64)   # [L, 128]
        acc += part.reshape(L, 2, B).sum(axis=1).T
    return acc.astype(np.float32)
